# revision 27
# baseline (speedup 1.0000x reference)
"""Multi-head attention (B=4, S=2048, D=1024, H=16, causal) on 8 TRN2 NeuronCores.

Sharding: core i handles batch i//2 and head-group i%2 (8 heads / 512 projection
columns). Each core computes a partial output projection over its 512 rows of Wo;
the host sums the two partials per batch and adds bo. No device collectives.

Per-core dataflow (bf16 matmuls, fp32 softmax):
  QT/KT = W-stationary projections of pre-transposed x; V in natural layout with
  an interleaved ones column per head (softmax denominator rides the AV matmul).
  Scores are computed transposed [k, q] in 3-k-tile PSUM chunks; one wide ACT
  exp per chunk evicts to SBUF bf16; causal masking is a single multiply per
  diagonal k-tile against a host-provided mask; AV accumulates [out^T | denom];
  normalization uses a DMA-reshaped reciprocal ([1,512] -> [128,4] so the DVE
  divides 4 elements per lane instead of 512) and a GPSIMD partition broadcast.
"""

import sys

for _p in ("/opt/trn_rl_repo",):
    if _p not in sys.path:
        sys.path.insert(0, _p)

import numpy as np
import ml_dtypes

BF16 = ml_dtypes.bfloat16

B, S, D = 4, 2048, 1024
H, HD = 16, 64
HPC = H // 2          # heads per core: 8
DPC = D // 2          # projection cols per core: 512
NCORES = 8
SCALE = 1.0 / np.sqrt(np.float32(HD))
CH = 3                # k-tiles per score chunk (3 PSUM banks, double buffered)

_compiled = None


def _chunks(nkt):
    out, s = [], 0
    while s < nkt:
        n = min(CH, nkt - s)
        out.append((s, n))
        s += n
    return out


def _build():
    import concourse.bacc as bacc
    import concourse.mybir as mybir
    import concourse.tile as tile

    f32 = mybir.dt.float32
    bf = mybir.dt.bfloat16
    Exp = mybir.ActivationFunctionType.Exp
    Copy = mybir.ActivationFunctionType.Copy

    nc = bacc.Bacc("TRN2", target_bir_lowering=False, debug=False)

    xtq = nc.dram_tensor("xtq", [D, S], bf, kind="ExternalInput")
    xtk = nc.dram_tensor("xtk", [D, S], bf, kind="ExternalInput")
    xtv = nc.dram_tensor("xtv", [D, S], bf, kind="ExternalInput")
    wq = nc.dram_tensor("wq", [D, DPC], bf, kind="ExternalInput")
    wk = nc.dram_tensor("wk", [D, DPC], bf, kind="ExternalInput")
    wv = nc.dram_tensor("wv", [D, DPC], bf, kind="ExternalInput")
    wo = nc.dram_tensor("wo", [DPC, D], bf, kind="ExternalInput")
    bq = nc.dram_tensor("bq", [1, DPC], bf, kind="ExternalInput")
    bk = nc.dram_tensor("bk", [1, DPC], bf, kind="ExternalInput")
    bv = nc.dram_tensor("bv", [1, DPC], bf, kind="ExternalInput")
    dmask = nc.dram_tensor("dmask", [128, 2048], bf, kind="ExternalInput")
    y = nc.dram_tensor("y", [S, D], f32, kind="ExternalOutput")

    NKD = D // 128        # 8 contraction tiles for projections
    NST = S // 128        # 16 seq tiles
    NSB = S // 512        # 4 seq blocks
    NHP = HPC // 2        # 4 head pairs / 128-wide col groups

    with tile.TileContext(nc) as tc:
        with (
            tc.tile_pool(name="consts", bufs=1) as consts,
            tc.tile_pool(name="wqp", bufs=NKD) as wqp,
            tc.tile_pool(name="wkp", bufs=NKD) as wkp,
            tc.tile_pool(name="wvp", bufs=NKD) as wvp,
            tc.tile_pool(name="wop", bufs=4) as wop,
            tc.tile_pool(name="xt", bufs=1) as xtp,
            tc.tile_pool(name="qt", bufs=NHP) as qtp,
            tc.tile_pool(name="kt", bufs=NHP) as ktp,
            tc.tile_pool(name="vp", bufs=NST) as vpool,
            tc.tile_pool(name="ex", bufs=3) as expool,
            tc.tile_pool(name="ot", bufs=NHP) as otp,
            tc.tile_pool(name="ys", bufs=2) as ysp,
            tc.tile_pool(name="rb", bufs=1) as rbp,
            tc.tile_pool(name="rc", bufs=1) as rcp,
            tc.tile_pool(name="ps", bufs=2, space="PSUM") as psp,
            tc.tile_pool(name="sc", bufs=2, space="PSUM") as scp,
        ):
            # constants
            dmt = consts.tile([128, 2048], bf, tag="dmt")
            nc.sync.dma_start(dmt[:], dmask.ap()[:])
            ones = consts.tile([1, 512], bf, tag="ones")
            nc.gpsimd.memset(ones[:], 1.0)
            bqt = consts.tile([1, DPC], bf, tag="bq")
            nc.sync.dma_start(bqt[:], bq.ap()[:])
            bkt = consts.tile([1, DPC], bf, tag="bk")
            nc.sync.dma_start(bkt[:], bk.ap()[:])
            bvt = consts.tile([1, DPC], bf, tag="bv")
            nc.sync.dma_start(bvt[:], bv.ap()[:])

            # PE warmup: junk matmuls while input DMAs land, so HAM ramps to
            # full clock before the first real projection group
            warm = consts.tile([128, 512], bf, tag="warm")
            nc.gpsimd.memset(warm[:], 0.25)
            wps = psp.tile([128, 512], f32, name="wps", tag="ps")
            for _ in range(40):
                nc.tensor.matmul(wps[:], warm[:, 0:128], warm[:], start=True, stop=True)

            # weights: wv + xtv queued first so the first V matmul starts ASAP.
            # x inputs are loaded as [128, 512] quarters: SBUF slot reuse then
            # pairs xtk[kd][sb] with xtv[kd][sb], whose readers finish after
            # only 4 V groups -- without this, the xtk DMAs wait for the WHOLE
            # V projection and serialize ~70us of the kernel.
            wvt = []
            for kd in range(NKD):
                w = wvp.tile([128, DPC], bf, name=f"wv{kd}", tag="wv")
                nc.sync.dma_start(w[:], wv.ap()[kd * 128:(kd + 1) * 128, :])
                wvt.append(w)

            def make_quarter(src_t, prefix, tagp, kd, sb):
                xt = xtp.tile([128, 512], bf, name=f"{prefix}{kd}_{sb}",
                              tag=f"{tagp}{kd}_{sb}", bufs=1)
                nc.sync.dma_start(
                    xt[:],
                    src_t.ap()[kd * 128:(kd + 1) * 128, sb * 512:(sb + 1) * 512])
                return xt

            # quarters load in need order: sb=0 of everything first, so the
            # first V / QT0 / KT0 groups start after ~6MB instead of ~16MB
            xtv_q = [[None] * NSB for _ in range(NKD)]
            xtq_q = [[None] * NSB for _ in range(NKD)]
            xtk_q = [[None] * NSB for _ in range(NKD)]
            for kd in range(NKD):
                xtv_q[kd][0] = make_quarter(xtv, "xv", "xvk", kd, 0)
            wqt, wkt = [], []
            for kd in range(NKD):
                for lst, pool, t, nm in ((wqt, wqp, wq, "wq"), (wkt, wkp, wk, "wk")):
                    w = pool.tile([128, DPC], bf, name=f"{nm}{kd}", tag=nm)
                    nc.sync.dma_start(w[:], t.ap()[kd * 128:(kd + 1) * 128, :])
                    lst.append(w)
            for kd in range(NKD):
                xtq_q[kd][0] = make_quarter(xtq, "xq", "xq", kd, 0)
            for kd in range(NKD):
                xtk_q[kd][0] = make_quarter(xtk, "xk", "xvk", kd, 0)
            for sb in range(1, NSB):
                for kd in range(NKD):
                    xtv_q[kd][sb] = make_quarter(xtv, "xv", "xvk", kd, sb)
                for kd in range(NKD):
                    xtq_q[kd][sb] = make_quarter(xtq, "xq", "xq", kd, sb)
                for kd in range(NKD):
                    xtk_q[kd][sb] = make_quarter(xtk, "xk", "xvk", kd, sb)
            wot = []
            for hp in range(4):
                w = wop.tile([128, D], bf, name=f"wo{hp}", tag="wo")
                nc.sync.dma_start(w[:], wo.ap()[hp * 128:(hp + 1) * 128, :])
                wot.append(w)

            # ---- V projection groups (natural layout, [8 heads x 65] + ones)
            vts = [vpool.tile([128, HPC * 65], bf, name=f"v{st}", tag="v")
                   for st in range(NST)]

            def v_group(st):
                def group():
                    ps = psp.tile([128, 512], f32, name="psv", tag="ps")
                    for kd in range(NKD):
                        nc.tensor.matmul(
                            ps[:],
                            xtv_q[kd][st // 4][:, (st % 4) * 128:(st % 4 + 1) * 128],
                            wvt[kd][:],
                            start=(kd == 0), stop=False,
                        )
                    nc.tensor.matmul(ps[:], ones[0:1, 0:128], bvt[0:1, :],
                                     start=False, stop=True)
                    vt = vts[st]
                    v3 = vt[:].rearrange("p (h c) -> p h c", h=HPC, c=65)
                    nc.vector.tensor_copy(
                        v3[:, :, 0:64],
                        ps[:].rearrange("p (h c) -> p h c", h=HPC, c=64),
                    )
                    nc.gpsimd.memset(v3[:, :, 64:65], 1.0)
                return group

            # ---- QT / KT projection groups
            qts, kts = [], []
            for pool, lst, nm in ((qtp, qts, "qt"), (ktp, kts, "kt")):
                for hp in range(NHP):
                    lst.append(pool.tile([128, S], bf, name=f"{nm}{hp}", tag=nm))

            def proj_group(xq, wts, bias, dest, hp, sb):
                def group():
                    ps = psp.tile([128, 512], f32, name="psq", tag="ps")
                    for kd in range(NKD):
                        nc.tensor.matmul(
                            ps[:],
                            wts[kd][:, hp * 128:(hp + 1) * 128],
                            xq[kd][sb][:],
                            start=(kd == 0), stop=False,
                        )
                    nc.tensor.matmul(
                        ps[:],
                        bias[0:1, hp * 128:(hp + 1) * 128],
                        ones[0:1, :],
                        start=False, stop=True,
                    )
                    nc.vector.tensor_copy(dest[:, sb * 512:(sb + 1) * 512], ps[:])
                return group

            # upfront: V st0-3 + sb0 of QT0/KT0; everything else is filler
            for st in range(4):
                v_group(st)()
            proj_group(xtq_q, wqt, bqt, qts[0], 0, 0)()
            proj_group(xtk_q, wkt, bkt, kts[0], 0, 0)()

            filler = []
            for j in range(1, NSB):
                for st in range(4 * j, 4 * j + 4):
                    filler.append(v_group(st))
                filler.append(proj_group(xtq_q, wqt, bqt, qts[0], 0, j))
                filler.append(proj_group(xtk_q, wkt, bkt, kts[0], 0, j))
            for hp in range(1, NHP):
                for sb in range(NSB):
                    filler.append(proj_group(xtq_q, wqt, bqt, qts[hp], hp, sb))
                for sb in range(NSB):
                    filler.append(proj_group(xtk_q, wkt, bkt, kts[hp], hp, sb))
            emitted = [0]

            def pop_filler_until(n):
                while emitted[0] < min(n, len(filler)):
                    filler[emitted[0]]()
                    emitted[0] += 1

            def need(h, j):
                if h == 0:
                    return 6 * j
                if h == 1:
                    return 18
                return 18 + 8 * (h // 2)

            ots = [otp.tile([128, S], bf, name=f"ot{i}", tag="ot") for i in range(NHP)]

            def yproj_group(st, eb):
                def group():
                    ps = psp.tile([128, 512], f32, name="psy", tag="ps")
                    for hp in range(NHP):
                        nc.tensor.matmul(
                            ps[:],
                            ots[hp][:, st * 128:(st + 1) * 128],
                            wot[hp][:, eb * 512:(eb + 1) * 512],
                            start=(hp == 0), stop=(hp == NHP - 1),
                        )
                    ys = ysp.tile([128, 512], f32, name="ys", tag="ys")
                    if (st + eb) % 2 == 0:
                        nc.vector.tensor_copy(ys[:], ps[:])
                    else:
                        nc.scalar.activation(ys[:], ps[:], Copy)
                    nc.sync.dma_start(
                        y.ap()[st * 128:(st + 1) * 128, eb * 512:(eb + 1) * 512],
                        ys[:],
                    )
                return group

            # ---- attention: scoresT [k, q] chunks of CH k-tiles, AV delayed one
            # chunk (software pipeline) so PE never waits on the exp of the
            # chunk it just scored. Projection/yproj groups are woven in as
            # whole-group filler to keep the tensor engine HAM-warm.
            proj_chunks = sum(len(_chunks(4 * (j + 1))) for j in range(NSB)) * 6
            pace = max(1, proj_chunks // max(1, len(filler)))
            chunk_no = [0]
            yfiller = []

            def maybe_filler():
                if emitted[0] < len(filler) and chunk_no[0] % pace == 0:
                    pop_filler_until(emitted[0] + 1)
                elif yfiller:
                    yfiller.pop()()
                    if len(yfiller) > 4:
                        yfiller.pop()()

            def attend(h, j):
                pop_filler_until(need(h, j))
                hp, sub = h // 2, h % 2
                base = sub * 64
                qt_h = qts[hp][base:base + 64, :]
                kt_h = kts[hp][base:base + 64, :]
                av = psp.tile([128, 512], f32, name="av", tag="ps")
                nkt = 4 * (j + 1)
                # diagonal k-tiles first: their mask multiplies overlap with
                # later chunks instead of sitting on the (h, j) critical tail
                kt_order = list(range(4 * j, nkt)) + list(range(0, 4 * j))
                prev_av = None

                def make_av(ex, kts_c, first):
                    def emit():
                        for r, kti in enumerate(kts_c):
                            nc.tensor.matmul(
                                av[0:65, :],
                                vts[kti][:, h * 65:(h + 1) * 65],
                                ex[:, r * 512:(r + 1) * 512],
                                start=(first and r == 0),
                                stop=(kti == kt_order[-1] and r == len(kts_c) - 1),
                            )
                    return emit

                first = True
                for (c0, cn) in _chunks(nkt):
                    kts_c = kt_order[c0:c0 + cn]
                    sc = scp.tile([128, CH * 512], f32, name="sc")
                    for r, kti in enumerate(kts_c):
                        nc.tensor.matmul(
                            sc[:, r * 512:(r + 1) * 512],
                            kt_h[:, kti * 128:(kti + 1) * 128],
                            qt_h[:, j * 512:(j + 1) * 512],
                            start=True, stop=True,
                        )
                    ex = expool.tile([128, CH * 512], bf, name="ex")
                    nc.scalar.activation(
                        ex[:, 0:cn * 512], sc[:, 0:cn * 512], Exp,
                        scale=float(SCALE))
                    for r, kti in enumerate(kts_c):
                        rr = kti - 4 * j
                        if rr >= 0:   # diagonal k-tile: causal mask multiply
                            nc.vector.tensor_mul(
                                ex[:, r * 512:(r + 1) * 512],
                                ex[:, r * 512:(r + 1) * 512],
                                dmt[:, rr * 512:(rr + 1) * 512],
                            )
                    chunk_no[0] += 1
                    maybe_filler()
                    if prev_av is not None:
                        prev_av()
                    prev_av = make_av(ex, kts_c, first)
                    first = False
                prev_av()
                # evict av to SBUF (frees PSUM slot), then normalize:
                # denom -> [128,4] reshape -> fast recip -> bcast -> multiply
                avs = ysp.tile([65, 512], f32, name="avs", tag="ys")
                nc.vector.tensor_copy(avs[:], av[0:65, :])
                rsh = rcp.tile([128, 4], f32, name="rsh", tag="rsh")
                nc.gpsimd.dma_start(rsh[:], avs[64:65, :])
                rr_t = rcp.tile([128, 4], f32, name="rr", tag="rr")
                nc.vector.reciprocal(rr_t[:], rsh[:])
                rrow = rcp.tile([1, 512], f32, name="rrow", tag="rrow")
                nc.gpsimd.dma_start(rrow[:], rr_t[:])
                rb = rbp.tile([64, 512], f32, name="rb", tag="rb")
                nc.gpsimd.partition_broadcast(rb[:], rrow[:], channels=64)
                nc.vector.tensor_mul(
                    ots[hp][base:base + 64, j * 512:(j + 1) * 512],
                    avs[0:64, :],
                    rb[:],
                )

            for h in range(6):
                for j in range(NSB):
                    attend(h, j)
            # last head pair: j-major; yproj tiles become filler two j's after
            # their ot slices were written, so the normalize chains have
            # executed (not merely been emitted) by the time PE reaches them
            yhold = []
            for j in range(NSB):
                attend(6, j)
                attend(7, j)
                yfiller.extend(yhold)
                yhold = [yproj_group(st, eb)
                         for st in range(4 * j, 4 * j + 4) for eb in range(2)]
            pop_filler_until(len(filler))
            for g in yfiller + yhold:
                g()

    nc.compile()
    return nc


def _diag_mask():
    tri = np.triu(np.ones((128, 128), np.float32))  # mask[k,q]=1 iff k<=q
    m = np.ones((128, 2048), np.float32)
    for r in range(4):
        m[:, r * 512:r * 512 + r * 128] = 0.0
        m[:, r * 512 + r * 128:r * 512 + (r + 1) * 128] = tri
    return m.astype(BF16)


def _shard_inputs(q_in, k_in, v_in, Wq, bq, Wk, bk, Wv, bv, Wo, bo):
    dm = _diag_mask()
    in_maps = []
    for core in range(NCORES):
        b, g = core // 2, core % 2
        cs = slice(g * DPC, (g + 1) * DPC)
        in_maps.append({
            "xtq": np.ascontiguousarray(q_in[b].T).astype(BF16),
            "xtk": np.ascontiguousarray(k_in[b].T).astype(BF16),
            "xtv": np.ascontiguousarray(v_in[b].T).astype(BF16),
            "wq": Wq[:, cs].astype(BF16),
            "wk": Wk[:, cs].astype(BF16),
            "wv": Wv[:, cs].astype(BF16),
            "wo": np.ascontiguousarray(Wo[cs, :]).astype(BF16),
            "bq": bq[cs].reshape(1, DPC).astype(BF16),
            "bk": bk[cs].reshape(1, DPC).astype(BF16),
            "bv": bv[cs].reshape(1, DPC).astype(BF16),
            "dmask": dm,
        })
    return in_maps


def kernel(q_in, k_in, v_in, Wq, bq, Wk, bk, Wv, bv, Wo, bo, _trace=False):
    from concourse.bass_utils import run_bass_kernel_spmd

    global _compiled
    if _compiled is None:
        _compiled = _build()

    args = [np.asarray(a, np.float32) for a in
            (q_in, k_in, v_in, Wq, bq, Wk, bk, Wv, bv, Wo, bo)]
    in_maps = _shard_inputs(*args)
    res = run_bass_kernel_spmd(
        _compiled, in_maps, core_ids=list(range(NCORES)), trace=_trace,
    )
    bo_f = args[10]
    out = np.empty((B, S, D), np.float32)
    for b in range(B):
        out[b] = res.results[2 * b]["y"] + res.results[2 * b + 1]["y"] + bo_f
    if _trace:
        kernel.last_results = res
    return out
